# revision 14
# baseline (speedup 1.0000x reference)
import os
import sys

import numpy as np

for _p in ("/opt/trn_rl_repo", "/root/.axon_site/_ro/trn_rl_repo"):
    if _p not in sys.path and os.path.isdir(_p):
        sys.path.append(_p)

B = 128
S = 8192
VOCAB = 5
EMBED = 128
N_CORES = 8
ROWS = B // N_CORES
CHUNKS = 8
CHUNK = S // CHUNKS
SPAD = S + 1
XW = CHUNK + 1
T0 = 768
H0 = T0 + 1

_STATE = {}


def _params(emb_table, lin_w, lin_b):
    emb = np.asarray(emb_table, np.float64)
    lw = np.asarray(lin_w, np.float64).reshape(-1)
    bias = float(np.asarray(lin_b, np.float64).reshape(-1)[0])
    p = emb @ lw[:EMBED]
    c = emb @ lw[EMBED:] + bias

    t = np.arange(VOCAB, dtype=np.float64)
    V = np.vander(t, VOCAB, increasing=True)

    def quartic(vals):
        a = np.linalg.solve(V, vals)
        if abs(a[4]) < 1e-7:
            vals = vals + 1e-6 * np.array([1.0, -4.0, 6.0, -4.0, 1.0])
            a = np.linalg.solve(V, vals)
        return a

    ap = quartic(p)
    alp = ap[3] / (4 * ap[4])
    c0 = ap[2] / (2 * ap[4]) - 2 * alp * alp
    qp = c0 - alp * alp
    rp = ap[1] - 4 * ap[4] * alp * c0
    cp = ap[0] - ap[4] * c0 * c0
    sgp = 1.0 if ap[4] > 0 else -1.0
    sp = np.sqrt(abs(ap[4]))
    bp = qp * sp

    ac = quartic(c)
    alc = ac[3] / (2 * ac[4])
    qc = (ac[2] / ac[4] - alc * alc) / 2
    rc = ac[1] - 2 * ac[4] * alc * qc
    cc = ac[0] - ac[4] * qc * qc
    sgc = 1.0 if ac[4] > 0 else -1.0
    sc = np.sqrt(abs(ac[4]))
    bc = qc * sc

    K = cp + cc
    alc2 = alc / 2
    bc2 = bc - sc * alc2 * alc2
    f = float
    return dict(alp=f(alp), sp=f(sp), bp=f(bp), sgp=f(sgp), rp=f(rp),
                alc2=f(alc2), sc=f(sc), bc2=f(bc2), sgc=f(sgc), rc=f(rc),
                K=f(K))


def _build_nc(P):
    import concourse.bass as bass
    import concourse.mybir as mybir
    from concourse.ap import AP

    f32 = mybir.dt.float32
    f16 = mybir.dt.float16
    u8 = mybir.dt.uint8
    MUL = mybir.AluOpType.mult
    ADD = mybir.AluOpType.add
    SUB = mybir.AluOpType.subtract
    SQ = mybir.ActivationFunctionType.Square

    sgp, sgc = P["sgp"], P["sgc"]
    ra_s1, ra_s2 = P["rp"] * sgp, P["K"] * sgp
    rb_s1 = P["rc"] * sgc

    nc = bass.Bass()
    x_ext = nc.dram_tensor("xin", [ROWS, SPAD], u8, kind="ExternalInput")
    y_ext = nc.dram_tensor("yout", [ROWS, S], f16, kind="ExternalOutput")

    x_srcA = AP(x_ext, 0, [[SPAD, ROWS], [CHUNK, CHUNKS], [1, H0]])
    x_srcB = AP(x_ext, H0, [[SPAD, ROWS], [CHUNK, CHUNKS], [1, XW - H0]])
    y_dst = y_ext[:, :].rearrange("r (c j) -> (r c) j", j=CHUNK)

    X = nc.alloc_sbuf_tensor("X", [128, XW], u8)
    W = nc.alloc_sbuf_tensor("W", [128, CHUNK], f16)
    GA = nc.alloc_sbuf_tensor("GA", [128, CHUNK], f16)
    YA = nc.alloc_sbuf_tensor("YA", [128, CHUNK], f16)
    YB = nc.alloc_sbuf_tensor("YB", [128, CHUNK], f16)
    SB = nc.alloc_sbuf_tensor("SB", [128, CHUNK], f16)
    RA = nc.alloc_sbuf_tensor("RA", [128, CHUNK], f16)
    RB = nc.alloc_sbuf_tensor("RB", [128, CHUNK], f16)
    V1 = nc.alloc_sbuf_tensor("V1", [128, CHUNK], f16)
    V2 = nc.alloc_sbuf_tensor("V2", [128, CHUNK], f16)
    OUT = nc.alloc_sbuf_tensor("OUT", [128, CHUNK], f16)
    BIAS = nc.alloc_sbuf_tensor("BIAS", [128, 3], f32)
    DUMMY = nc.alloc_sbuf_tensor("DUMMY", [128, 1], f16)

    dsemA = nc.alloc_semaphore("dsemA")
    dsemB = nc.alloc_semaphore("dsemB")
    vsem = nc.alloc_semaphore("vsem")
    asem = nc.alloc_semaphore("asem")
    psem = nc.alloc_semaphore("psem")
    osem = nc.alloc_semaphore("osem")

    TBOUND = [0, T0, CHUNK]

    def tile(t, i):
        return t[:, TBOUND[i]:TBOUND[i + 1]]

    def _emit_out(vector, i):
        if both_neg:
            vector.tensor_tensor(out=tile(V1, i), in0=tile(V1, i),
                                 in1=tile(V2, i), op=ADD).then_inc(vsem, 1)
            vector.tensor_scalar(out=tile(OUT, i), in0=tile(V1, i),
                                 scalar1=-1.0, scalar2=0.0,
                                 op0=MUL, op1=ADD).then_inc(vsem, 1)
        elif sgp > 0 and sgc > 0:
            vector.tensor_tensor(out=tile(OUT, i), in0=tile(V1, i),
                                 in1=tile(V2, i), op=ADD).then_inc(vsem, 1)
        elif sgp > 0:
            vector.tensor_tensor(out=tile(OUT, i), in0=tile(V1, i),
                                 in1=tile(V2, i), op=SUB).then_inc(vsem, 1)
        else:
            vector.tensor_tensor(out=tile(OUT, i), in0=tile(V2, i),
                                 in1=tile(V1, i), op=SUB).then_inc(vsem, 1)

    both_neg = sgp < 0 and sgc < 0
    out_ready = [10, 13] if not both_neg else [11, 15]

    nc.sync.dma_start(X[:, 0:H0], x_srcA).then_inc(dsemA, 16)
    nc.sync.dma_start(X[:, H0:XW], x_srcB).then_inc(dsemB, 16)

    with nc.Block(no_gpsimd_drain=True) as block:

        @block.gpsimd
        def _(gpsimd):
            gpsimd.wait_ge(dsemA, 16)
            gpsimd.tensor_scalar(out=tile(RA, 0), in0=X[:, 0:T0],
                                 scalar1=ra_s1, scalar2=ra_s2,
                                 op0=MUL, op1=ADD).then_inc(psem, 1)
            gpsimd.tensor_scalar(out=tile(RB, 0), in0=X[:, 1:T0 + 1],
                                 scalar1=rb_s1, scalar2=0.0,
                                 op0=MUL, op1=ADD).then_inc(psem, 1)
            gpsimd.wait_ge(dsemB, 16)
            gpsimd.tensor_scalar(out=tile(RA, 1), in0=X[:, T0:CHUNK],
                                 scalar1=ra_s1, scalar2=ra_s2,
                                 op0=MUL, op1=ADD).then_inc(psem, 1)
            gpsimd.tensor_scalar(out=tile(RB, 1), in0=X[:, T0 + 1:XW],
                                 scalar1=rb_s1, scalar2=0.0,
                                 op0=MUL, op1=ADD).then_inc(psem, 1)
        @block.sync
        def _(sync):
            sync.wait_ge(vsem, out_ready[0])
            sync.dma_start(y_dst[:, 0:T0], tile(OUT, 0)).then_inc(osem, 16)
            sync.wait_ge(vsem, out_ready[1])
            sync.dma_start(y_dst[:, T0:CHUNK], tile(OUT, 1)).then_inc(osem, 16)

        @block.scalar
        def _(scalar):
            const0 = nc.const_aps.tensor(0.0, (128, 1), f32)
            scalar.activation(out=DUMMY[:], in_=const0, func=SQ,
                              bias=0.0, scale=1.0)
            scalar.wait_ge(vsem, 3)
            scalar.wait_ge(dsemA, 16)
            scalar.activation(out=tile(GA, 0), in_=X[:, 0:T0], func=SQ,
                              bias=BIAS[:, 0:1], scale=1.0).then_inc(asem, 1)
            scalar.wait_ge(asem, 1)
            scalar.activation(out=tile(YA, 0), in_=tile(GA, 0), func=SQ,
                              bias=BIAS[:, 1:2], scale=P["sp"]).then_inc(asem, 1)
            scalar.wait_ge(vsem, 6)
            scalar.activation(out=tile(YB, 0), in_=tile(SB, 0), func=SQ,
                              bias=BIAS[:, 2:3], scale=P["sc"]).then_inc(asem, 1)
            scalar.wait_ge(dsemB, 16)
            scalar.activation(out=tile(GA, 1), in_=X[:, T0:CHUNK], func=SQ,
                              bias=BIAS[:, 0:1], scale=1.0).then_inc(asem, 1)
            scalar.wait_ge(asem, 4)
            scalar.activation(out=tile(YA, 1), in_=tile(GA, 1), func=SQ,
                              bias=BIAS[:, 1:2], scale=P["sp"]).then_inc(asem, 1)
            scalar.wait_ge(vsem, 7)
            scalar.activation(out=tile(YB, 1), in_=tile(SB, 1), func=SQ,
                              bias=BIAS[:, 2:3], scale=P["sc"]).then_inc(asem, 1)

        @block.vector
        def _(vector):
            vector.memset(BIAS[:, 0:1], P["alp"]).then_inc(vsem, 1)
            vector.memset(BIAS[:, 1:2], P["bp"]).then_inc(vsem, 1)
            vector.memset(BIAS[:, 2:3], P["bc2"]).then_inc(vsem, 1)
            vector.wait_ge(dsemA, 16)
            vector.tensor_scalar(out=tile(W, 0), in0=X[:, 1:T0 + 1],
                                 scalar1=1.0, scalar2=P["alc2"],
                                 op0=MUL, op1=ADD).then_inc(vsem, 1)
            vector.wait_ge(dsemB, 16)
            vector.tensor_scalar(out=tile(W, 1), in0=X[:, T0 + 1:XW],
                                 scalar1=1.0, scalar2=P["alc2"],
                                 op0=MUL, op1=ADD).then_inc(vsem, 1)
            vector.wait_ge(vsem, 5)
            vector.tensor_tensor(out=tile(SB, 0), in0=tile(W, 0),
                                 in1=tile(W, 0), op=MUL).then_inc(vsem, 1)
            vector.wait_ge(vsem, 6)
            vector.tensor_tensor(out=tile(SB, 1), in0=tile(W, 1),
                                 in1=tile(W, 1), op=MUL).then_inc(vsem, 1)
            vector.wait_ge(asem, 2)
            vector.wait_ge(psem, 1)
            vector.tensor_tensor(out=tile(V1, 0), in0=tile(YA, 0),
                                 in1=tile(RA, 0), op=ADD).then_inc(vsem, 1)
            vector.wait_ge(asem, 3)
            vector.wait_ge(psem, 2)
            vector.tensor_tensor(out=tile(V2, 0), in0=tile(YB, 0),
                                 in1=tile(RB, 0), op=ADD).then_inc(vsem, 1)
            vector.wait_ge(vsem, 9)
            _emit_out(vector, 0)
            vector.wait_ge(asem, 5)
            vector.wait_ge(psem, 3)
            vector.tensor_tensor(out=tile(V1, 1), in0=tile(YA, 1),
                                 in1=tile(RA, 1), op=ADD).then_inc(vsem, 1)
            vector.wait_ge(asem, 6)
            vector.wait_ge(psem, 4)
            vector.tensor_tensor(out=tile(V2, 1), in0=tile(YB, 1),
                                 in1=tile(RB, 1), op=ADD).then_inc(vsem, 1)
            vector.wait_ge(vsem, 12 if not both_neg else 13)
            _emit_out(vector, 1)

    return nc



def _get_nc(P):
    key = tuple(sorted(P.items()))
    if _STATE.get("key") != key:
        _STATE["nc"] = _build_nc(P)
        _STATE["key"] = key
    return _STATE["nc"]


def _run(x, emb_table, lin_w, lin_b, trace=False):
    from concourse.bass_utils import run_bass_kernel_spmd

    P = _params(emb_table, lin_w, lin_b)

    xq = np.asarray(x)
    assert xq.shape == (B, S), xq.shape
    xpad = np.zeros((B, SPAD), np.uint8)
    xpad[:, :S] = xq.astype(np.uint8)

    in_maps = [
        {"xin": np.ascontiguousarray(xpad[ROWS * i:ROWS * (i + 1)])}
        for i in range(N_CORES)
    ]
    nc = _get_nc(P)
    res = run_bass_kernel_spmd(nc, in_maps, list(range(N_CORES)), trace=trace)
    y = np.concatenate([res.results[i]["yout"] for i in range(N_CORES)], axis=0)
    return np.ascontiguousarray(y[:, :S - 1]).astype(np.float32), res


def kernel(x, emb_table, lin_w, lin_b):
    y, _ = _run(x, emb_table, lin_w, lin_b, trace=False)
    return y
